# revision 38
# baseline (speedup 1.0000x reference)
"""AttentionalPooler Trainium2 kernel (v2: software-pipelined).

Full inputs -> full outputs; internally data-parallel over batch across 8
NeuronCores (b=8, one batch element per core).

Per-core math (one batch element; matmuls bf16, everything else fp32):
  xk  = LN(x)                      [4096, 1024]
  q   = (LN(query) @ Wq) * scale   [256, 1024]   (identical on every core)
  kT  = Wk'^T @ xk^T               [1024, 4096]  (K stored transposed)
  V   = xk @ Wv'                   [4096, 1024]  (row-major, +ones col/head)
  S^T = kT_h^T-slices @ qT_h       [4096, 256] per head  (j on partitions)
  E   = exp(S^T)  (no max subtraction; |S| <= ~8.2 so fp32/bf16-safe)
  [O^T_h; den_h] = [V_h | 1]^T @ E  accumulated over j   [65, 256]
  out = sum_h (O_h / den_h) @ Wout_h                     [256, 1024]

v2 schedule: the projection work for quarter q+1 (LN, transposes, kT, V —
TensorE/DVE heavy) is emitted interleaved with the attention work of
quarter q (ST/exp/OT — the exp is ScalarE-bound), so the ScalarE exp
latency hides under TensorE projection work instead of serializing.
kT/v buffers are double-buffered across quarters.  LN's rsqrt runs as a
Newton iteration on DVE (keeps ScalarE's activation table pinned to Exp),
and K-projection PSUM evacuations run on ScalarE as Copy.

LN gamma and the attention scale are folded into the weights host-side;
LN beta becomes a bias vector applied at PSUM evacuation.
"""

import os
import sys
import types

for _p in ("/root/.axon_site", "/root/.axon_site/_ro/trn_rl_repo", "/opt/trn_rl_repo"):
    if os.path.isdir(_p) and _p not in sys.path:
        sys.path.append(_p)

# The image's antenv package lacks axon_hooks; shim it with the ctypes-based
# NTFF hook from trn_agent_boot so trace=True works under axon.
try:
    import antenv.axon_hooks  # noqa: F401
except ImportError:
    try:
        import trn_agent_boot.trn_boot as _tb

        _hook = _tb._ntff_profile_via_ctypes("/opt/axon/libaxon_pjrt.so")
    except Exception:
        _hook = None
    _m = types.ModuleType("antenv.axon_hooks")
    _m.get_axon_ntff_profile_hook = lambda: _hook
    sys.modules["antenv.axon_hooks"] = _m

import numpy as np

import concourse.bass as bass
import concourse.tile as tile
from concourse import mybir
from concourse.masks import make_identity

D = 1024          # model dim == ctx dim
NCTX = 4096       # keys per batch element
NQ = 256          # queries
H = 16            # heads
DH = 64           # head dim
NCORES = 8
EPS = 1e-5
QTR = 1024        # keys per quarter (4 quarters)
SUP = 512         # kT-projection moving-dim tile (2 supers per quarter)
NQTR = NCTX // QTR
MAGIC = 0x5F3759DF

F32 = mybir.dt.float32
I32 = mybir.dt.int32
BF16 = mybir.dt.bfloat16
MM_DT = BF16
ALU = mybir.AluOpType


def _mm_np():
    if MM_DT == F32:
        return np.float32
    import ml_dtypes

    return ml_dtypes.bfloat16


def _patch_drain(max_waits=1):
    """This walrus build rejects >1 sync-wait on the SP Drain that Tile emits
    at kernel exit. Split the waits across a chain of drains."""

    def patched(self, tick_clock, wait_clock):
        from concourse.vector_clock import ScopedClock

        drain_inst = self.nc.sync.drain()
        wait_clock.add_sem_waits(
            drain_inst.ins, ScopedClock({None: tick_clock.global_clock})
        )
        si = drain_inst.ins.sync_info
        waits = list(si.on_wait or []) if si else []
        if len(waits) > max_waits:
            si.on_wait = waits[:max_waits]
            rest = waits[max_waits:]
            while rest:
                extra = self.nc.sync.drain()
                extra.ins.sync_info = mybir.SyncInfo(
                    on_wait=rest[:max_waits], on_update=[]
                )
                rest = rest[max_waits:]
        self.nc.all_engine_barrier()
        assert self.sems is not None
        popped = self.nc._tile_sem_poison_stack.pop()
        assert popped is self._sem_poison
        self.nc.clear_and_free_semaphores(list(self.sems.allocated().values()))
        self.nc.all_engine_barrier()

    tile.TileContext._drain_and_barrier = patched


_patch_drain()


def _split_sync_waits(nc, max_waits=1):
    """This walrus build rejects instructions carrying more than one sync
    wait. Hoist excess waits onto same-engine NoOps placed just before the
    owning instruction (engine queues are serial, so this is equivalent)."""
    for f in nc.m.functions:
        for bb in f.blocks:
            new_list = []
            changed = False
            for inst in bb.instructions:
                si = inst.sync_info
                waits = list(si.on_wait) if si and si.on_wait else []
                if len(waits) > max_waits:
                    changed = True
                    keep = waits[-max_waits:]
                    rest = waits[:-max_waits]
                    k = 0
                    while rest:
                        carrier = mybir.InstNoOp(
                            name=f"{inst.name}-w{k}", ins=[], outs=[]
                        )
                        carrier.engine = inst.engine
                        carrier.sync_info = mybir.SyncInfo(
                            on_wait=rest[:max_waits], on_update=[]
                        )
                        rest = rest[max_waits:]
                        k += 1
                        nc.register_instruction(carrier, overwrite=True)
                        new_list.append(carrier)
                    si.on_wait = keep
                new_list.append(inst)
            if changed:
                bb.instructions = new_list


class _Builder:
    """Holds all build state so emission units can be freely interleaved."""

    def __init__(self, nc, pools, io):
        self.nc = nc
        self.p = pools
        self.io = io
        self.xt_handles = {}       # tile index -> xt sbuf tile
        self.xkT = {}              # (q, s) -> xkT tile
        self.kT_q = {}             # q -> kT tile
        self.v_q = {}              # q -> v tile
        self.psos = {}             # hc -> (pso0, pso1)
        self.ets = {}              # (hc, jjp) -> (et0, et1)
        self.wo_tiles = {}         # head-group -> wo sbuf tile
        self.denbs = {}            # head-group -> 1/den bf16 tile
        self.st_pool = "ps_st"     # PSUM pool for S^T tiles (swapped in tail)
        # x-tile order: 32 tiles, (q, s, jt)
        self.tiles = [
            (q, s, jt) for q in range(NQTR) for s in range(2) for jt in range(4)
        ]

    # ---------------- DMA prefetch ----------------
    def xtile_dma(self, n):
        if n >= len(self.tiles) or n in self.xt_handles:
            return
        q, s, jt = self.tiles[n]
        j0 = q * QTR + s * SUP + jt * 128
        xt = self.p["xpool"].tile([128, D], F32, tag="xt", name=f"xt{n}")
        if n < 3:
            # split across 4 queues: a single 512KB queue transfer has ~20us
            # latency, which would stall the pipeline head at startup
            for k in range(4):
                self.nc.sync.dma_start(
                    out=xt[32 * k:32 * (k + 1), :],
                    in_=self.io["x"][j0 + 32 * k:j0 + 32 * (k + 1), :],
                )
        else:
            self.nc.sync.dma_start(out=xt, in_=self.io["x"][j0:j0 + 128, :])
        self.xt_handles[n] = xt

    # ---------------- LN (stats + newton rsqrt + apply) ----------------
    def ln_apply(self, xt, xnb):
        nc, per = self.nc, self.p["per"]
        stats = per.tile([128, 2, nc.vector.BN_STATS_DIM], F32, tag="stats")
        for sg in range(2):
            nc.vector.bn_stats(
                out=stats[:, sg, :], in_=xt[:, sg * 512:(sg + 1) * 512]
            )
        mv = per.tile([128, nc.vector.BN_AGGR_DIM], F32, tag="mv")
        nc.vector.bn_aggr(out=mv, in_=stats)
        t = per.tile([128, 1], F32, tag="nt")
        nc.vector.tensor_scalar(
            out=t, in0=mv[:, 1:2], scalar1=EPS, scalar2=None, op0=ALU.add
        )
        sh = per.tile([128, 1], I32, tag="nsh")
        nc.vector.tensor_scalar(
            out=sh, in0=t.bitcast(I32), scalar1=1, scalar2=None,
            op0=ALU.logical_shift_right,
        )
        ya = per.tile([128, 1], F32, tag="nya")
        yb = per.tile([128, 1], F32, tag="nyb")
        y2 = per.tile([128, 1], F32, tag="ny2")
        u = per.tile([128, 1], F32, tag="nu")
        nc.vector.scalar_tensor_tensor(
            out=ya.bitcast(I32), in0=sh, scalar=-1, in1=self.magic_t,
            op0=ALU.mult, op1=ALU.add,
        )
        rstd = per.tile([128, 1], F32, tag="nrstd")
        for src, dst in ((ya, yb), (yb, rstd)):
            nc.vector.tensor_tensor(out=y2, in0=src, in1=src, op=ALU.mult)
            nc.vector.scalar_tensor_tensor(
                out=u, in0=y2, scalar=-0.5, in1=t, op0=ALU.mult, op1=ALU.mult
            )
            nc.vector.scalar_tensor_tensor(
                out=dst, in0=u, scalar=1.5, in1=src, op0=ALU.add, op1=ALU.mult
            )
        nc.vector.tensor_scalar(
            out=xnb, in0=xt, scalar1=mv[:, 0:1], scalar2=rstd,
            op0=ALU.subtract, op1=ALU.mult,
        )

    # ---------------- A units (projection work for one quarter) ----------
    def unit_xtile(self, n):
        """LN + 8 transposes for x tile n; prefetches tile n+3's DMA."""
        nc = self.nc
        self.xtile_dma(n + 3)
        q, s, jt = self.tiles[n]
        xt = self.xt_handles.pop(n)
        xnb = self.p["xpool"].tile([128, D], MM_DT, tag="xnb", bufs=2)
        self.ln_apply(xt, xnb)
        xkT = self.xkT[(q, s)]
        for c in range(4):
            ptr = self.p["ps_tr"].tile([128, 2, 128], MM_DT, tag="tr")
            for k in range(2):
                dc = c * 2 + k
                nc.tensor.transpose(
                    ptr[:, k, :], xnb[:, dc * 128:(dc + 1) * 128], self.identb
                )
            nc.vector.tensor_copy(
                out=xkT[:, c * 2:c * 2 + 2, jt * 128:(jt + 1) * 128], in_=ptr
            )

    def unit_v(self, q, s, jt, nt):
        """V rows for x-tile (s, jt), output half nt (512 e-cols)."""
        nc = self.nc
        xkT = self.xkT[(q, s)]
        psv = self.p["ps_mm"].tile([128, SUP], F32, tag="mm")
        for dc in range(8):
            nc.tensor.matmul(
                psv,
                lhsT=xkT[:, dc, jt * 128:(jt + 1) * 128],
                rhs=self.wv_sb[:, dc, nt * 512:(nt + 1) * 512],
                start=(dc == 0), stop=(dc == 7),
            )
        jj = s * 4 + jt
        vdst = self.v_q[q][:, jj, nt * 8 * 65:(nt + 1) * 8 * 65].rearrange(
            "p (h c) -> p h c", c=65
        )[:, :, 0:64]
        nc.vector.tensor_copy(
            out=vdst, in_=psv.rearrange("p (h c) -> p h c", c=64)
        )

    def unit_kt(self, q, s, ec):
        """kT e-chunk ec for super s (512 j columns); evacuated on ScalarE."""
        nc = self.nc
        xkT = self.xkT[(q, s)]
        psk = self.p["ps_mm"].tile([128, SUP], F32, tag="mm")
        for dc in range(8):
            nc.tensor.matmul(
                psk,
                lhsT=self.wk_sb[:, dc, ec * 128:(ec + 1) * 128],
                rhs=xkT[:, dc, :],
                start=(dc == 0), stop=(dc == 7),
            )
        nc.scalar.copy(
            out=self.kT_q[q][:, ec, s * SUP:(s + 1) * SUP], in_=psk
        )

    def open_quarter(self, q):
        """Allocate double-buffered kT/v/xkT tiles for quarter q."""
        big, xkp = self.p["big"], self.p["xkp"]
        self.kT_q[q] = big.tile(
            [128, 8, QTR], MM_DT, tag="kt", bufs=2, name=f"kT{q}"
        )
        vq = big.tile(
            [128, QTR // 128, H * 65], MM_DT, tag="vq", bufs=2, name=f"v{q}"
        )
        self.v_q[q] = vq
        ones_view = vq.rearrange("p j (h c) -> p j h c", c=65)[:, :, :, 64:65]
        self.nc.vector.memset(ones_view, 1.0)
        for s in range(2):
            self.xkT[(q, s)] = xkp.tile(
                [128, 8, SUP], MM_DT, tag="xkT", bufs=2, name=f"xkT{q}_{s}"
            )

    def a_units(self, q):
        """The 40 projection units for quarter q, in dependency order."""
        units = []
        for s in range(2):
            for jt in range(4):
                n = q * 8 + s * 4 + jt
                units.append(lambda n=n: self.unit_xtile(n))
                units.append(lambda q=q, s=s, jt=jt: self.unit_v(q, s, jt, 0))
                units.append(lambda q=q, s=s, jt=jt: self.unit_v(q, s, jt, 1))
            for ec in range(8):
                units.append(lambda q=q, s=s, ec=ec: self.unit_kt(q, s, ec))
        return units

    # ---------------- B chunks (attention for one quarter) ---------------
    def chunk_st(self, q, hc, jjp):
        """S^T for j-chunks 2*jjp, 2*jjp+1 of heads 2hc, 2hc+1 + exp."""
        nc = self.nc
        pool = self.st_pool
        pstp0 = self.p[pool].tile([128, 2, NQ], F32, tag=pool[3:], name="pstp0")
        pstp1 = self.p[pool].tile([128, 2, NQ], F32, tag=pool[3:], name="pstp1")
        pstps = (pstp0, pstp1)
        kT = self.kT_q[q]
        for u in range(2):
            jj = jjp * 2 + u
            for par in range(2):
                pb = par * 64
                nc.tensor.matmul(
                    pstps[par][:, u, :],
                    lhsT=kT[pb:pb + 64, hc, jj * 128:(jj + 1) * 128],
                    rhs=self.qT[pb:pb + 64, hc, :],
                    start=True, stop=True,
                )
        ets = []
        for par in range(2):
            et = self.p["etp"].tile([128, 2, NQ], MM_DT, tag="et")
            nc.scalar.activation(
                out=et, in_=pstps[par], func=mybir.ActivationFunctionType.Exp
            )
            ets.append(et)
        self.ets[(hc, jjp)] = ets

    def chunk_ot(self, q, hc, jjp):
        """O^T accumulation for j-chunk pair jjp of heads 2hc, 2hc+1."""
        nc = self.nc
        ets = self.ets.pop((hc, jjp))
        njj = QTR // 128
        for u in range(2):
            jj = jjp * 2 + u
            for par in range(2):
                h = hc * 2 + par
                nc.tensor.matmul(
                    self.psos[hc][par],
                    lhsT=self.v_q[q][:, jj, h * 65:(h + 1) * 65],
                    rhs=ets[par][:, u, :],
                    start=(jj == 0), stop=(jj == njj - 1),
                )

    def chunk_flush(self, q, hc):
        nc = self.nc
        psos = self.psos.pop(hc)
        for k in range(2):
            h = hc * 2 + k
            if q == 0:
                nc.vector.tensor_copy(out=self.otacc[:, h, :], in_=psos[k])
            else:
                nc.vector.tensor_add(
                    out=self.otacc[:, h, :], in0=self.otacc[:, h, :],
                    in1=psos[k],
                )

    def b_chunks(self, q, hc):
        """Chunk list for (q, hc); interleave A units at the None slots."""

        def c_open():
            self.open_psos(hc, "ps_ot")
            self.chunk_st(q, hc, 0)

        return [
            c_open,
            None,
            lambda: self.chunk_st(q, hc, 1),
            None,
            lambda: (self.chunk_ot(q, hc, 0), self.chunk_st(q, hc, 2)),
            None,
            lambda: (self.chunk_ot(q, hc, 1), self.chunk_st(q, hc, 3)),
            None,
            lambda: self.chunk_ot(q, hc, 2),
            None,
            lambda: (self.chunk_ot(q, hc, 3), self.chunk_flush(q, hc)),
        ]

    def open_psos(self, hc, pool):
        pso0 = self.p[pool].tile([65, NQ], F32, tag=pool[3:], name="pso0")
        pso1 = self.p[pool].tile([65, NQ], F32, tag=pool[3:], name="pso1")
        self.psos[hc] = (pso0, pso1)

    # ---------------- tail: normalize + output projection ----------------
    def normalize_head(self, h):
        """1/den then O_h/den_h -> ot_n for one head (den in otacc row 64)."""
        nc = self.nc
        nc.vector.reciprocal(
            out=self.otacc[64:65, h, :], in_=self.otacc[64:65, h, :]
        )
        denb = self.p["per"].tile([1, NQ], MM_DT, tag="denb", bufs=2)
        nc.vector.tensor_copy(out=denb, in_=self.otacc[64:65, h, :])
        psb = self.p["ps_tr"].tile([64, NQ], F32, tag="tr", name="psb")
        nc.tensor.matmul(
            psb, lhsT=self.ones_b[0:1, :], rhs=denb, start=True, stop=True
        )
        nc.vector.tensor_mul(
            out=self.ot_n[:, h, :], in0=self.otacc[0:64, h, :], in1=psb
        )

    def wo_dma(self, g):
        """Stream Wout head-group g%8 (2 heads); g>=8 is the second pass."""
        if g >= 16 or g in self.wo_tiles:
            return
        wo_r = self.io["wo"].rearrange("(g h p) f -> p g h f", p=64, h=2)
        wt = self.p["wstream"].tile([64, 2, D], MM_DT, tag="wos")
        self.nc.gpsimd.dma_start(out=wt, in_=wo_r[:, g % 8, :, :])
        self.wo_tiles[g] = wt

    def out_proj_head(self, h, ic):
        """Accumulate head h's contribution to output row-block ic."""
        nc = self.nc
        g = h // 2 + 8 * ic
        self.wo_dma(g + 1)
        wt = self.wo_tiles[g]
        if h % 2 == 1:
            self.wo_tiles.pop(g)
        for ft in range(2):
            nc.tensor.matmul(
                self.psf[2 * ic + ft],
                lhsT=self.ot_n[:, h, ic * 128:(ic + 1) * 128],
                rhs=wt[:, h % 2, ft * 512:(ft + 1) * 512],
                start=(h == 0), stop=(h == 15),
            )

    def out_flush(self, ic):
        nc = self.nc
        ot = self.p["outp"].tile([128, D], F32, tag="outsb")
        for ft in range(2):
            nc.scalar.copy(
                out=ot[:, ft * 512:(ft + 1) * 512], in_=self.psf[2 * ic + ft]
            )
        nc.sync.dma_start(
            out=self.io["out"][ic * 128:(ic + 1) * 128, :], in_=ot
        )

    def tail_units(self):
        """Filler for the final window: per-head normalize + the ic=0
        output-projection pass, gated on the producing hc's flush.  The
        ic=1 pass and drains run post-loop.  psf0/1 live in ps_mm, which has
        no other users in the final window; psb cycles ps_tr likewise."""
        self.psf = [
            self.p["ps_mm"].tile([128, 512], F32, tag="mm", name="psf0"),
            self.p["ps_mm"].tile([128, 512], F32, tag="mm", name="psf1"),
            None,
            None,
        ]
        units = [(-1, lambda: self.wo_dma(0))]
        for h in range(H):
            units.append((h // 2 + 1, lambda h=h: self.normalize_head(h)))
            units.append((h // 2 + 1, lambda h=h: self.out_proj_head(h, 0)))
        return units

    def tail_finish(self):
        self.out_flush(0)
        self.psf[2] = self.p["ps_mm"].tile([128, 512], F32, tag="mm",
                                           name="psf2")
        self.psf[3] = self.p["ps_mm"].tile([128, 512], F32, tag="mm",
                                           name="psf3")
        for h in range(H):
            self.out_proj_head(h, 1)
        self.out_flush(1)


def _build_body(nc, tc, io):
    import contextlib

    ctx = contextlib.ExitStack()
    with ctx:
        pools = {}
        for name, bufs, space in (
            ("consts", 1, "SBUF"),
            ("wpool", 1, "SBUF"),
            ("wstream", 2, "SBUF"),
            ("xpool", 4, "SBUF"),
            ("xkp", 2, "SBUF"),
            ("big", 1, "SBUF"),
            ("per", 3, "SBUF"),
            ("etp", 6, "SBUF"),
            ("outp", 2, "SBUF"),
            ("ps_mm", 2, "PSUM"),
            ("ps_st", 2, "PSUM"),
            ("ps_tr", 2, "PSUM"),
            ("ps_ot", 2, "PSUM"),
        ):
            pools[name] = ctx.enter_context(
                tc.tile_pool(name=name, bufs=bufs, space=space)
            )

        b = _Builder(nc, pools, io)
        consts, wpool = pools["consts"], pools["wpool"]

        # ---- constants / weights ----
        b.identb = consts.tile([128, 128], MM_DT, tag="identb", name="identb")
        make_identity(nc, b.identb)
        b.magic_t = consts.tile([128, 1], I32, tag="magic", name="magic")
        nc.vector.memset(b.magic_t, MAGIC)
        b.ones_b = consts.tile([128, 64], MM_DT, tag="onesb", name="onesb")
        nc.vector.memset(b.ones_b, 1.0)
        bq_sb = consts.tile([128, 8], F32, tag="bq", name="bqsb")
        nc.sync.dma_start(out=bq_sb, in_=io["bq"])

        # x prefetch ahead of the weight DMAs
        for n in range(3):
            b.xtile_dma(n)
        qts = []
        for t in range(2):
            qt = pools["xpool"].tile([128, D], F32, tag="xt", name=f"qt{t}")
            nc.sync.dma_start(out=qt, in_=io["qry"][t * 128:(t + 1) * 128, :])
            qts.append(qt)

        # resident k/v weights, chunked so early kT matmuls can start as
        # soon as their d-chunk lands
        wk_r = io["wk"].rearrange("(c p) e -> p c e", p=128)
        wv_r = io["wv"].rearrange("(c p) e -> p c e", p=128)
        b.wk_sb = wpool.tile([128, 8, D], MM_DT, tag="wk", name="wksb")
        for dc in range(8):
            nc.gpsimd.dma_start(out=b.wk_sb[:, dc, :], in_=wk_r[:, dc, :])
        b.wv_sb = wpool.tile([128, 8, D], MM_DT, tag="wv", name="wvsb")
        for dc in range(8):
            nc.gpsimd.dma_start(out=b.wv_sb[:, dc, :], in_=wv_r[:, dc, :])

        # ---- prologue: A(0) first super, then q-phase, then second super
        b.open_quarter(0)
        a0 = b.a_units(0)
        for u in a0[:20]:
            u()

        # q = LN(query) @ Wq' + bq, stored transposed as qT [e', ec, i]
        qnT = pools["xkp"].tile([128, 8, NQ], MM_DT, tag="qnT", name="qnT",
                                bufs=1)
        for t in range(2):
            qnb = pools["xpool"].tile([128, D], MM_DT, tag="xnb", bufs=2)
            b.ln_apply(qts[t], qnb)
            for c in range(4):
                ptr = pools["ps_tr"].tile([128, 2, 128], MM_DT, tag="tr")
                for k in range(2):
                    dc = c * 2 + k
                    nc.tensor.transpose(
                        ptr[:, k, :], qnb[:, dc * 128:(dc + 1) * 128], b.identb
                    )
                nc.vector.tensor_copy(
                    out=qnT[:, c * 2:c * 2 + 2, t * 128:(t + 1) * 128],
                    in_=ptr,
                )
        b.qT = consts.tile([128, 8, NQ], MM_DT, tag="qT", name="qT")
        wq_r = io["wq"].rearrange("(c p) e -> p c e", p=128)
        for ec in range(8):
            wq_t = pools["wstream"].tile([128, 8, 128], MM_DT, tag="wqs")
            nc.sync.dma_start(
                out=wq_t, in_=wq_r[:, :, ec * 128:(ec + 1) * 128]
            )
            psq = pools["ps_mm"].tile([128, NQ], F32, tag="mm")
            for dc in range(8):
                nc.tensor.matmul(
                    psq, lhsT=wq_t[:, dc, :], rhs=qnT[:, dc, :],
                    start=(dc == 0), stop=(dc == 7),
                )
            nc.vector.tensor_scalar(
                out=b.qT[:, ec, :], in0=psq, scalar1=bq_sb[:, ec:ec + 1],
                scalar2=None, op0=ALU.add,
            )
        for u in a0[20:]:
            u()

        # accumulators: [O^T_h ; den_h] per head, accumulated over quarters
        b.otacc = pools["big"].tile([65, H, NQ], F32, tag="ot", name="otacc")
        b.ot_n = pools["big"].tile([64, H, NQ], MM_DT, tag="otn", name="otn")

        # ---- main loop: B(q) interleaved with A(q+1) (or the tail) ----
        for q in range(NQTR):
            if q + 1 < NQTR:
                b.open_quarter(q + 1)
                filler = [(-1, u) for u in b.a_units(q + 1)]
            else:
                filler = b.tail_units()
            fi = 0
            for hc in range(8):
                for c in b.b_chunks(q, hc):
                    if c is None:
                        if fi < len(filler) and filler[fi][0] <= hc:
                            filler[fi][1]()
                            fi += 1
                    else:
                        c()
            while fi < len(filler):
                filler[fi][1]()
                fi += 1
        b.tail_finish()


def build_program():
    nc = bass.Bass("TRN2", target_bir_lowering=False, debug=False)

    io = {
        "x": nc.dram_tensor("x", [NCTX, D], F32, kind="ExternalInput").ap(),
        "qry": nc.dram_tensor("qry", [NQ, D], F32, kind="ExternalInput").ap(),
        "wq": nc.dram_tensor("wq", [D, D], MM_DT, kind="ExternalInput").ap(),
        "wk": nc.dram_tensor("wk", [D, D], MM_DT, kind="ExternalInput").ap(),
        "wv": nc.dram_tensor("wv", [D, D], MM_DT, kind="ExternalInput").ap(),
        "wo": nc.dram_tensor("wo", [D, D], MM_DT, kind="ExternalInput").ap(),
        "bq": nc.dram_tensor("bq", [128, 8], F32, kind="ExternalInput").ap(),

        "out": nc.dram_tensor("out", [NQ, D], F32, kind="ExternalOutput").ap(),
    }

    with tile.TileContext(nc) as tc:
        _build_body(nc, tc, io)
    _split_sync_waits(nc)
    return nc


_CACHED = None


def _get_program():
    global _CACHED
    if _CACHED is None:
        _CACHED = build_program()
    return _CACHED


def _prep_inputs(x, query, Wq, Wkv, Wout, ln_q_g, ln_q_b, ln_k_g, ln_k_b):
    scale = DH ** -0.5
    f32 = np.float32
    Wq = np.asarray(Wq, f32)
    Wkv = np.asarray(Wkv, f32)
    Wout = np.asarray(Wout, f32)
    wq_eff = (np.asarray(ln_q_g, f32)[:, None] * Wq * scale).astype(f32)
    bq_eff = (np.asarray(ln_q_b, f32) @ Wq * scale).astype(f32)
    wk_eff = (np.asarray(ln_k_g, f32)[:, None] * Wkv[:, :D]).astype(f32)
    bk_eff = (np.asarray(ln_k_b, f32) @ Wkv[:, :D]).astype(f32)
    wv_eff = (np.asarray(ln_k_g, f32)[:, None] * Wkv[:, D:]).astype(f32)
    bv_eff = (np.asarray(ln_k_b, f32) @ Wkv[:, D:]).astype(f32)
    assert np.abs(bk_eff).max() == 0.0, "nonzero k-bias needs the DVE evac path"
    assert np.abs(bv_eff).max() == 0.0, "nonzero v-bias needs the bias evac path"
    mdt = _mm_np()
    shared = {
        "qry": np.ascontiguousarray(np.asarray(query, f32)),
        "wq": np.ascontiguousarray(wq_eff.astype(mdt)),
        "wk": np.ascontiguousarray(wk_eff.astype(mdt)),
        "wv": np.ascontiguousarray(wv_eff.astype(mdt)),
        "wo": np.ascontiguousarray(Wout.astype(mdt)),
        "bq": np.ascontiguousarray(bq_eff.reshape(8, 128).T),
    }
    x = np.asarray(x, f32)
    in_maps = [
        dict(shared, x=np.ascontiguousarray(x[i])) for i in range(NCORES)
    ]
    return in_maps


def run(trace=False, **inputs):
    from concourse.bass_utils import run_bass_kernel_spmd

    nc = _get_program()
    in_maps = _prep_inputs(**inputs)
    res = run_bass_kernel_spmd(
        nc, in_maps, core_ids=list(range(NCORES)), trace=trace
    )
    out = np.stack([res.results[i]["out"] for i in range(NCORES)], axis=0)
    return out.astype(np.float32), res.exec_time_ns


def kernel(**inputs):
    out, _ = run(trace=False, **inputs)
    return out


# revision 41
# speedup vs baseline: 1.0010x; 1.0010x over previous
"""AttentionalPooler Trainium2 kernel (v2: software-pipelined).

Full inputs -> full outputs; internally data-parallel over batch across 8
NeuronCores (b=8, one batch element per core).

Per-core math (one batch element; matmuls bf16, everything else fp32):
  xk  = LN(x)                      [4096, 1024]
  q   = (LN(query) @ Wq) * scale   [256, 1024]   (identical on every core)
  kT  = Wk'^T @ xk^T               [1024, 4096]  (K stored transposed)
  V   = xk @ Wv'                   [4096, 1024]  (row-major, +ones col/head)
  S^T = kT_h^T-slices @ qT_h       [4096, 256] per head  (j on partitions)
  E   = exp(S^T)  (no max subtraction; |S| <= ~8.2 so fp32/bf16-safe)
  [O^T_h; den_h] = [V_h | 1]^T @ E  accumulated over j   [65, 256]
  out = sum_h (O_h / den_h) @ Wout_h                     [256, 1024]

v2 schedule: the projection work for quarter q+1 (LN, transposes, kT, V —
TensorE/DVE heavy) is emitted interleaved with the attention work of
quarter q (ST/exp/OT — the exp is ScalarE-bound), so the ScalarE exp
latency hides under TensorE projection work instead of serializing.
kT/v buffers are double-buffered across quarters.  LN's rsqrt runs as a
Newton iteration on DVE (keeps ScalarE's activation table pinned to Exp),
and K-projection PSUM evacuations run on ScalarE as Copy.

LN gamma and the attention scale are folded into the weights host-side;
LN beta becomes a bias vector applied at PSUM evacuation.
"""

import os
import sys
import types

for _p in ("/root/.axon_site", "/root/.axon_site/_ro/trn_rl_repo", "/opt/trn_rl_repo"):
    if os.path.isdir(_p) and _p not in sys.path:
        sys.path.append(_p)

# The image's antenv package lacks axon_hooks; shim it with the ctypes-based
# NTFF hook from trn_agent_boot so trace=True works under axon.
try:
    import antenv.axon_hooks  # noqa: F401
except ImportError:
    try:
        import trn_agent_boot.trn_boot as _tb

        _hook = _tb._ntff_profile_via_ctypes("/opt/axon/libaxon_pjrt.so")
    except Exception:
        _hook = None
    _m = types.ModuleType("antenv.axon_hooks")
    _m.get_axon_ntff_profile_hook = lambda: _hook
    sys.modules["antenv.axon_hooks"] = _m

import numpy as np

import concourse.bass as bass
import concourse.tile as tile
from concourse import mybir
from concourse.masks import make_identity

D = 1024          # model dim == ctx dim
NCTX = 4096       # keys per batch element
NQ = 256          # queries
H = 16            # heads
DH = 64           # head dim
NCORES = 8
EPS = 1e-5
QTR = 512         # keys per pipeline stage (8 stages)
SUP = 512         # kT-projection moving-dim tile (1 super per stage)
NQTR = NCTX // QTR
MAGIC = 0x5F3759DF

F32 = mybir.dt.float32
I32 = mybir.dt.int32
BF16 = mybir.dt.bfloat16
MM_DT = BF16
ALU = mybir.AluOpType


def _mm_np():
    if MM_DT == F32:
        return np.float32
    import ml_dtypes

    return ml_dtypes.bfloat16


def _patch_drain(max_waits=1):
    """This walrus build rejects >1 sync-wait on the SP Drain that Tile emits
    at kernel exit. Split the waits across a chain of drains."""

    def patched(self, tick_clock, wait_clock):
        from concourse.vector_clock import ScopedClock

        drain_inst = self.nc.sync.drain()
        wait_clock.add_sem_waits(
            drain_inst.ins, ScopedClock({None: tick_clock.global_clock})
        )
        si = drain_inst.ins.sync_info
        waits = list(si.on_wait or []) if si else []
        if len(waits) > max_waits:
            si.on_wait = waits[:max_waits]
            rest = waits[max_waits:]
            while rest:
                extra = self.nc.sync.drain()
                extra.ins.sync_info = mybir.SyncInfo(
                    on_wait=rest[:max_waits], on_update=[]
                )
                rest = rest[max_waits:]
        self.nc.all_engine_barrier()
        assert self.sems is not None
        popped = self.nc._tile_sem_poison_stack.pop()
        assert popped is self._sem_poison
        self.nc.clear_and_free_semaphores(list(self.sems.allocated().values()))
        self.nc.all_engine_barrier()

    tile.TileContext._drain_and_barrier = patched


_patch_drain()


def _split_sync_waits(nc, max_waits=1):
    """This walrus build rejects instructions carrying more than one sync
    wait. Hoist excess waits onto same-engine NoOps placed just before the
    owning instruction (engine queues are serial, so this is equivalent)."""
    for f in nc.m.functions:
        for bb in f.blocks:
            new_list = []
            changed = False
            for inst in bb.instructions:
                si = inst.sync_info
                waits = list(si.on_wait) if si and si.on_wait else []
                if len(waits) > max_waits:
                    changed = True
                    keep = waits[-max_waits:]
                    rest = waits[:-max_waits]
                    k = 0
                    while rest:
                        carrier = mybir.InstNoOp(
                            name=f"{inst.name}-w{k}", ins=[], outs=[]
                        )
                        carrier.engine = inst.engine
                        carrier.sync_info = mybir.SyncInfo(
                            on_wait=rest[:max_waits], on_update=[]
                        )
                        rest = rest[max_waits:]
                        k += 1
                        nc.register_instruction(carrier, overwrite=True)
                        new_list.append(carrier)
                    si.on_wait = keep
                new_list.append(inst)
            if changed:
                bb.instructions = new_list


class _Builder:
    """Holds all build state so emission units can be freely interleaved."""

    def __init__(self, nc, pools, io):
        self.nc = nc
        self.p = pools
        self.io = io
        self.xt_handles = {}       # tile index -> xt sbuf tile
        self.xkT = {}              # (q, s) -> xkT tile
        self.kT_q = {}             # q -> kT tile
        self.v_q = {}              # q -> v tile
        self.psos = {}             # hc -> (pso0, pso1)
        self.ets = {}              # (hc, jjp) -> (et0, et1)
        self.wo_tiles = {}         # head-group -> wo sbuf tile
        self.denbs = {}            # head-group -> 1/den bf16 tile
        self.st_pool = "ps_st"     # PSUM pool for S^T tiles (swapped in tail)
        # x-tile order: 32 tiles, (q, s, jt)
        self.tiles = [
            (q, s, jt) for q in range(NQTR) for s in range(1) for jt in range(4)
        ]

    # ---------------- DMA prefetch ----------------
    def xtile_dma(self, n):
        if n >= len(self.tiles) or n in self.xt_handles:
            return
        q, s, jt = self.tiles[n]
        j0 = q * QTR + s * SUP + jt * 128
        xt = self.p["xpool"].tile([128, D], F32, tag="xt", name=f"xt{n}")
        if n < 3:
            # split across 4 queues: a single 512KB queue transfer has ~20us
            # latency, which would stall the pipeline head at startup
            for k in range(4):
                self.nc.sync.dma_start(
                    out=xt[32 * k:32 * (k + 1), :],
                    in_=self.io["x"][j0 + 32 * k:j0 + 32 * (k + 1), :],
                )
        else:
            self.nc.sync.dma_start(out=xt, in_=self.io["x"][j0:j0 + 128, :])
        self.xt_handles[n] = xt

    # ---------------- LN (stats + newton rsqrt + apply) ----------------
    def ln_apply(self, xt, xnb):
        nc, per = self.nc, self.p["per"]
        stats = per.tile([128, 2, nc.vector.BN_STATS_DIM], F32, tag="stats")
        for sg in range(2):
            nc.vector.bn_stats(
                out=stats[:, sg, :], in_=xt[:, sg * 512:(sg + 1) * 512]
            )
        mv = per.tile([128, nc.vector.BN_AGGR_DIM], F32, tag="mv")
        nc.vector.bn_aggr(out=mv, in_=stats)
        t = per.tile([128, 1], F32, tag="nt")
        nc.vector.tensor_scalar(
            out=t, in0=mv[:, 1:2], scalar1=EPS, scalar2=None, op0=ALU.add
        )
        sh = per.tile([128, 1], I32, tag="nsh")
        nc.vector.tensor_scalar(
            out=sh, in0=t.bitcast(I32), scalar1=1, scalar2=None,
            op0=ALU.logical_shift_right,
        )
        ya = per.tile([128, 1], F32, tag="nya")
        yb = per.tile([128, 1], F32, tag="nyb")
        y2 = per.tile([128, 1], F32, tag="ny2")
        u = per.tile([128, 1], F32, tag="nu")
        nc.vector.scalar_tensor_tensor(
            out=ya.bitcast(I32), in0=sh, scalar=-1, in1=self.magic_t,
            op0=ALU.mult, op1=ALU.add,
        )
        rstd = per.tile([128, 1], F32, tag="nrstd")
        for src, dst in ((ya, yb), (yb, rstd)):
            nc.vector.tensor_tensor(out=y2, in0=src, in1=src, op=ALU.mult)
            nc.vector.scalar_tensor_tensor(
                out=u, in0=y2, scalar=-0.5, in1=t, op0=ALU.mult, op1=ALU.mult
            )
            nc.vector.scalar_tensor_tensor(
                out=dst, in0=u, scalar=1.5, in1=src, op0=ALU.add, op1=ALU.mult
            )
        nc.vector.tensor_scalar(
            out=xnb, in0=xt, scalar1=mv[:, 0:1], scalar2=rstd,
            op0=ALU.subtract, op1=ALU.mult,
        )

    # ---------------- A units (projection work for one quarter) ----------
    def unit_xtile(self, n):
        """LN + 8 transposes for x tile n; prefetches tile n+3's DMA."""
        nc = self.nc
        self.xtile_dma(n + 3)
        q, s, jt = self.tiles[n]
        xt = self.xt_handles.pop(n)
        xnb = self.p["xpool"].tile([128, D], MM_DT, tag="xnb", bufs=2)
        self.ln_apply(xt, xnb)
        xkT = self.xkT[(q, s)]
        for c in range(4):
            ptr = self.p["ps_tr"].tile([128, 2, 128], MM_DT, tag="tr")
            for k in range(2):
                dc = c * 2 + k
                nc.tensor.transpose(
                    ptr[:, k, :], xnb[:, dc * 128:(dc + 1) * 128], self.identb
                )
            nc.vector.tensor_copy(
                out=xkT[:, c * 2:c * 2 + 2, jt * 128:(jt + 1) * 128], in_=ptr
            )

    def unit_v(self, q, s, jt, nt):
        """V rows for x-tile (s, jt), output half nt (512 e-cols)."""
        nc = self.nc
        xkT = self.xkT[(q, s)]
        psv = self.p["ps_mm"].tile([128, SUP], F32, tag="mm")
        for dc in range(8):
            nc.tensor.matmul(
                psv,
                lhsT=xkT[:, dc, jt * 128:(jt + 1) * 128],
                rhs=self.wv_sb[:, dc, nt * 512:(nt + 1) * 512],
                start=(dc == 0), stop=(dc == 7),
            )
        jj = s * 4 + jt
        vdst = self.v_q[q][:, jj, nt * 8 * 65:(nt + 1) * 8 * 65].rearrange(
            "p (h c) -> p h c", c=65
        )[:, :, 0:64]
        nc.vector.tensor_copy(
            out=vdst, in_=psv.rearrange("p (h c) -> p h c", c=64)
        )

    def unit_kt(self, q, s, ec):
        """kT e-chunk ec for super s (512 j columns); evacuated on ScalarE."""
        nc = self.nc
        xkT = self.xkT[(q, s)]
        psk = self.p["ps_mm"].tile([128, SUP], F32, tag="mm")
        for dc in range(8):
            nc.tensor.matmul(
                psk,
                lhsT=self.wk_sb[:, dc, ec * 128:(ec + 1) * 128],
                rhs=xkT[:, dc, :],
                start=(dc == 0), stop=(dc == 7),
            )
        nc.scalar.copy(
            out=self.kT_q[q][:, ec, s * SUP:(s + 1) * SUP], in_=psk
        )

    def open_quarter(self, q):
        """Allocate double-buffered kT/v/xkT tiles for quarter q."""
        big, xkp = self.p["big"], self.p["xkp"]
        self.kT_q[q] = big.tile(
            [128, 8, QTR], MM_DT, tag="kt", bufs=2, name=f"kT{q}"
        )
        vq = big.tile(
            [128, QTR // 128, H * 65], MM_DT, tag="vq", bufs=2, name=f"v{q}"
        )
        self.v_q[q] = vq
        ones_view = vq.rearrange("p j (h c) -> p j h c", c=65)[:, :, :, 64:65]
        self.nc.vector.memset(ones_view, 1.0)
        for s in range(1):
            self.xkT[(q, s)] = xkp.tile(
                [128, 8, SUP], MM_DT, tag="xkT", bufs=2, name=f"xkT{q}_{s}"
            )

    def a_units(self, q):
        """The 40 projection units for quarter q, in dependency order."""
        units = []
        for s in range(1):
            for jt in range(4):
                n = q * 4 + s * 4 + jt
                units.append(lambda n=n: self.unit_xtile(n))
                units.append(lambda q=q, s=s, jt=jt: self.unit_v(q, s, jt, 0))
                units.append(lambda q=q, s=s, jt=jt: self.unit_v(q, s, jt, 1))
            for ec in range(8):
                units.append(lambda q=q, s=s, ec=ec: self.unit_kt(q, s, ec))
        return units

    # ---------------- B chunks (attention for one quarter) ---------------
    def chunk_st(self, q, hc, jjp):
        """S^T for j-chunks 2*jjp, 2*jjp+1 of heads 2hc, 2hc+1 + exp."""
        nc = self.nc
        pool = self.st_pool
        pstp0 = self.p[pool].tile([128, 2, NQ], F32, tag=pool[3:], name="pstp0")
        pstp1 = self.p[pool].tile([128, 2, NQ], F32, tag=pool[3:], name="pstp1")
        pstps = (pstp0, pstp1)
        kT = self.kT_q[q]
        for u in range(2):
            jj = jjp * 2 + u
            for par in range(2):
                pb = par * 64
                nc.tensor.matmul(
                    pstps[par][:, u, :],
                    lhsT=kT[pb:pb + 64, hc, jj * 128:(jj + 1) * 128],
                    rhs=self.qT[pb:pb + 64, hc, :],
                    start=True, stop=True,
                )
        ets = []
        for par in range(2):
            et = self.p["etp"].tile([128, 2, NQ], MM_DT, tag="et")
            nc.scalar.activation(
                out=et, in_=pstps[par], func=mybir.ActivationFunctionType.Exp
            )
            ets.append(et)
        self.ets[(hc, jjp)] = ets

    def chunk_ot(self, q, hc, jjp):
        """O^T accumulation for j-chunk pair jjp of heads 2hc, 2hc+1."""
        nc = self.nc
        ets = self.ets.pop((hc, jjp))
        njj = QTR // 128
        for u in range(2):
            jj = jjp * 2 + u
            for par in range(2):
                h = hc * 2 + par
                nc.tensor.matmul(
                    self.psos[hc][par],
                    lhsT=self.v_q[q][:, jj, h * 65:(h + 1) * 65],
                    rhs=ets[par][:, u, :],
                    start=(jj == 0), stop=(jj == njj - 1),
                )

    def chunk_flush(self, q, hc):
        nc = self.nc
        psos = self.psos.pop(hc)
        for k in range(2):
            h = hc * 2 + k
            if q == 0:
                nc.vector.tensor_copy(out=self.otacc[:, h, :], in_=psos[k])
            else:
                nc.vector.tensor_add(
                    out=self.otacc[:, h, :], in0=self.otacc[:, h, :],
                    in1=psos[k],
                )

    def b_chunks(self, q, hc):
        """Chunk list for (q, hc); interleave A units at the None slots."""

        def c_open():
            self.open_psos(hc, "ps_ot")
            self.chunk_st(q, hc, 0)

        return [
            c_open,
            None,
            lambda: self.chunk_st(q, hc, 1),
            None,
            lambda: self.chunk_ot(q, hc, 0),
            None,
            lambda: (self.chunk_ot(q, hc, 1), self.chunk_flush(q, hc)),
        ]

    def open_psos(self, hc, pool):
        pso0 = self.p[pool].tile([65, NQ], F32, tag=pool[3:], name="pso0")
        pso1 = self.p[pool].tile([65, NQ], F32, tag=pool[3:], name="pso1")
        self.psos[hc] = (pso0, pso1)

    # ---------------- tail: normalize + output projection ----------------
    def normalize_head(self, h):
        """1/den then O_h/den_h -> ot_n for one head (den in otacc row 64)."""
        nc = self.nc
        nc.vector.reciprocal(
            out=self.otacc[64:65, h, :], in_=self.otacc[64:65, h, :]
        )
        denb = self.p["per"].tile([1, NQ], MM_DT, tag="denb", bufs=2)
        nc.vector.tensor_copy(out=denb, in_=self.otacc[64:65, h, :])
        psb = self.p["ps_tr"].tile([64, NQ], F32, tag="tr", name="psb")
        nc.tensor.matmul(
            psb, lhsT=self.ones_b[0:1, :], rhs=denb, start=True, stop=True
        )
        nc.vector.tensor_mul(
            out=self.ot_n[:, h, :], in0=self.otacc[0:64, h, :], in1=psb
        )

    def wo_dma(self, g):
        """Stream Wout head-group g%8 (2 heads); g>=8 is the second pass."""
        if g >= 16 or g in self.wo_tiles:
            return
        wo_r = self.io["wo"].rearrange("(g h p) f -> p g h f", p=64, h=2)
        wt = self.p["wstream"].tile([64, 2, D], MM_DT, tag="wos")
        self.nc.gpsimd.dma_start(out=wt, in_=wo_r[:, g % 8, :, :])
        self.wo_tiles[g] = wt

    def out_proj_head(self, h, ic):
        """Accumulate head h's contribution to output row-block ic."""
        nc = self.nc
        g = h // 2 + 8 * ic
        self.wo_dma(g + 1)
        wt = self.wo_tiles[g]
        if h % 2 == 1:
            self.wo_tiles.pop(g)
        for ft in range(2):
            nc.tensor.matmul(
                self.psf[2 * ic + ft],
                lhsT=self.ot_n[:, h, ic * 128:(ic + 1) * 128],
                rhs=wt[:, h % 2, ft * 512:(ft + 1) * 512],
                start=(h == 0), stop=(h == 15),
            )

    def out_flush(self, ic):
        nc = self.nc
        ot = self.p["outp"].tile([128, D], F32, tag="outsb")
        for ft in range(2):
            nc.scalar.copy(
                out=ot[:, ft * 512:(ft + 1) * 512], in_=self.psf[2 * ic + ft]
            )
        nc.sync.dma_start(
            out=self.io["out"][ic * 128:(ic + 1) * 128, :], in_=ot
        )

    def tail_units(self):
        """Filler for the final window: per-head normalize + the ic=0
        output-projection pass, gated on the producing hc's flush.  The
        ic=1 pass and drains run post-loop.  psf0/1 live in ps_mm, which has
        no other users in the final window; psb cycles ps_tr likewise."""
        self.psf = [
            self.p["ps_mm"].tile([128, 512], F32, tag="mm", name="psf0"),
            self.p["ps_mm"].tile([128, 512], F32, tag="mm", name="psf1"),
            None,
            None,
        ]
        units = [(-1, lambda: self.wo_dma(0))]
        for h in range(H):
            units.append((h // 2 + 1, lambda h=h: self.normalize_head(h)))
            units.append((h // 2 + 1, lambda h=h: self.out_proj_head(h, 0)))
        return units

    def tail_finish(self):
        self.out_flush(0)
        self.psf[2] = self.p["ps_mm"].tile([128, 512], F32, tag="mm",
                                           name="psf2")
        self.psf[3] = self.p["ps_mm"].tile([128, 512], F32, tag="mm",
                                           name="psf3")
        for h in range(H):
            self.out_proj_head(h, 1)
        self.out_flush(1)


def _build_body(nc, tc, io):
    import contextlib

    ctx = contextlib.ExitStack()
    with ctx:
        pools = {}
        for name, bufs, space in (
            ("consts", 1, "SBUF"),
            ("wpool", 1, "SBUF"),
            ("wstream", 2, "SBUF"),
            ("xpool", 4, "SBUF"),
            ("xkp", 2, "SBUF"),
            ("big", 1, "SBUF"),
            ("per", 3, "SBUF"),
            ("etp", 6, "SBUF"),
            ("outp", 2, "SBUF"),
            ("ps_mm", 2, "PSUM"),
            ("ps_st", 2, "PSUM"),
            ("ps_tr", 2, "PSUM"),
            ("ps_ot", 2, "PSUM"),
        ):
            pools[name] = ctx.enter_context(
                tc.tile_pool(name=name, bufs=bufs, space=space)
            )

        b = _Builder(nc, pools, io)
        consts, wpool = pools["consts"], pools["wpool"]

        # ---- constants / weights ----
        b.identb = consts.tile([128, 128], MM_DT, tag="identb", name="identb")
        make_identity(nc, b.identb)
        b.magic_t = consts.tile([128, 1], I32, tag="magic", name="magic")
        nc.vector.memset(b.magic_t, MAGIC)
        b.ones_b = consts.tile([128, 64], MM_DT, tag="onesb", name="onesb")
        nc.vector.memset(b.ones_b, 1.0)
        bq_sb = consts.tile([128, 8], F32, tag="bq", name="bqsb")
        nc.sync.dma_start(out=bq_sb, in_=io["bq"])

        # x prefetch ahead of the weight DMAs
        for n in range(3):
            b.xtile_dma(n)
        qts = []
        for t in range(2):
            qt = pools["xpool"].tile([128, D], F32, tag="xt", name=f"qt{t}")
            nc.sync.dma_start(out=qt, in_=io["qry"][t * 128:(t + 1) * 128, :])
            qts.append(qt)

        # resident k/v weights, chunked so early kT matmuls can start as
        # soon as their d-chunk lands
        wk_r = io["wk"].rearrange("(c p) e -> p c e", p=128)
        wv_r = io["wv"].rearrange("(c p) e -> p c e", p=128)
        b.wk_sb = wpool.tile([128, 8, D], MM_DT, tag="wk", name="wksb")
        for dc in range(8):
            nc.gpsimd.dma_start(out=b.wk_sb[:, dc, :], in_=wk_r[:, dc, :])
        b.wv_sb = wpool.tile([128, 8, D], MM_DT, tag="wv", name="wvsb")
        for dc in range(8):
            nc.gpsimd.dma_start(out=b.wv_sb[:, dc, :], in_=wv_r[:, dc, :])

        # ---- prologue: A(0) first super, then q-phase, then second super
        b.open_quarter(0)
        a0 = b.a_units(0)
        for u in a0[:12]:
            u()

        # q = LN(query) @ Wq' + bq, stored transposed as qT [e', ec, i]
        qnT = pools["xkp"].tile([128, 8, NQ], MM_DT, tag="qnT", name="qnT",
                                bufs=1)
        for t in range(2):
            qnb = pools["xpool"].tile([128, D], MM_DT, tag="xnb", bufs=2)
            b.ln_apply(qts[t], qnb)
            for c in range(4):
                ptr = pools["ps_tr"].tile([128, 2, 128], MM_DT, tag="tr")
                for k in range(2):
                    dc = c * 2 + k
                    nc.tensor.transpose(
                        ptr[:, k, :], qnb[:, dc * 128:(dc + 1) * 128], b.identb
                    )
                nc.vector.tensor_copy(
                    out=qnT[:, c * 2:c * 2 + 2, t * 128:(t + 1) * 128],
                    in_=ptr,
                )
        b.qT = consts.tile([128, 8, NQ], MM_DT, tag="qT", name="qT")
        wq_r = io["wq"].rearrange("(c p) e -> p c e", p=128)
        for ec in range(8):
            wq_t = pools["wstream"].tile([128, 8, 128], MM_DT, tag="wqs")
            nc.sync.dma_start(
                out=wq_t, in_=wq_r[:, :, ec * 128:(ec + 1) * 128]
            )
            psq = pools["ps_mm"].tile([128, NQ], F32, tag="mm")
            for dc in range(8):
                nc.tensor.matmul(
                    psq, lhsT=wq_t[:, dc, :], rhs=qnT[:, dc, :],
                    start=(dc == 0), stop=(dc == 7),
                )
            nc.vector.tensor_scalar(
                out=b.qT[:, ec, :], in0=psq, scalar1=bq_sb[:, ec:ec + 1],
                scalar2=None, op0=ALU.add,
            )
        for u in a0[12:]:
            u()

        # accumulators: [O^T_h ; den_h] per head, accumulated over quarters
        b.otacc = pools["big"].tile([65, H, NQ], F32, tag="ot", name="otacc")
        b.ot_n = pools["big"].tile([64, H, NQ], MM_DT, tag="otn", name="otn")

        # ---- main loop: B(q) interleaved with A(q+1) (or the tail) ----
        for q in range(NQTR):
            if q + 1 < NQTR:
                b.open_quarter(q + 1)
                filler = [(-1, u) for u in b.a_units(q + 1)]
            else:
                filler = b.tail_units()
            fi = 0
            for hc in range(8):
                for c in b.b_chunks(q, hc):
                    if c is None:
                        if fi < len(filler) and filler[fi][0] <= hc:
                            filler[fi][1]()
                            fi += 1
                    else:
                        c()
            while fi < len(filler):
                filler[fi][1]()
                fi += 1
        b.tail_finish()


def build_program():
    nc = bass.Bass("TRN2", target_bir_lowering=False, debug=False)

    io = {
        "x": nc.dram_tensor("x", [NCTX, D], F32, kind="ExternalInput").ap(),
        "qry": nc.dram_tensor("qry", [NQ, D], F32, kind="ExternalInput").ap(),
        "wq": nc.dram_tensor("wq", [D, D], MM_DT, kind="ExternalInput").ap(),
        "wk": nc.dram_tensor("wk", [D, D], MM_DT, kind="ExternalInput").ap(),
        "wv": nc.dram_tensor("wv", [D, D], MM_DT, kind="ExternalInput").ap(),
        "wo": nc.dram_tensor("wo", [D, D], MM_DT, kind="ExternalInput").ap(),
        "bq": nc.dram_tensor("bq", [128, 8], F32, kind="ExternalInput").ap(),

        "out": nc.dram_tensor("out", [NQ, D], F32, kind="ExternalOutput").ap(),
    }

    with tile.TileContext(nc) as tc:
        _build_body(nc, tc, io)
    _split_sync_waits(nc)
    return nc


_CACHED = None


def _get_program():
    global _CACHED
    if _CACHED is None:
        _CACHED = build_program()
    return _CACHED


def _prep_inputs(x, query, Wq, Wkv, Wout, ln_q_g, ln_q_b, ln_k_g, ln_k_b):
    scale = DH ** -0.5
    f32 = np.float32
    Wq = np.asarray(Wq, f32)
    Wkv = np.asarray(Wkv, f32)
    Wout = np.asarray(Wout, f32)
    wq_eff = (np.asarray(ln_q_g, f32)[:, None] * Wq * scale).astype(f32)
    bq_eff = (np.asarray(ln_q_b, f32) @ Wq * scale).astype(f32)
    wk_eff = (np.asarray(ln_k_g, f32)[:, None] * Wkv[:, :D]).astype(f32)
    bk_eff = (np.asarray(ln_k_b, f32) @ Wkv[:, :D]).astype(f32)
    wv_eff = (np.asarray(ln_k_g, f32)[:, None] * Wkv[:, D:]).astype(f32)
    bv_eff = (np.asarray(ln_k_b, f32) @ Wkv[:, D:]).astype(f32)
    assert np.abs(bk_eff).max() == 0.0, "nonzero k-bias needs the DVE evac path"
    assert np.abs(bv_eff).max() == 0.0, "nonzero v-bias needs the bias evac path"
    mdt = _mm_np()
    shared = {
        "qry": np.ascontiguousarray(np.asarray(query, f32)),
        "wq": np.ascontiguousarray(wq_eff.astype(mdt)),
        "wk": np.ascontiguousarray(wk_eff.astype(mdt)),
        "wv": np.ascontiguousarray(wv_eff.astype(mdt)),
        "wo": np.ascontiguousarray(Wout.astype(mdt)),
        "bq": np.ascontiguousarray(bq_eff.reshape(8, 128).T),
    }
    x = np.asarray(x, f32)
    in_maps = [
        dict(shared, x=np.ascontiguousarray(x[i])) for i in range(NCORES)
    ]
    return in_maps


def run(trace=False, **inputs):
    from concourse.bass_utils import run_bass_kernel_spmd

    nc = _get_program()
    in_maps = _prep_inputs(**inputs)
    res = run_bass_kernel_spmd(
        nc, in_maps, core_ids=list(range(NCORES)), trace=trace
    )
    out = np.stack([res.results[i]["out"] for i in range(NCORES)], axis=0)
    return out.astype(np.float32), res.exec_time_ns


def kernel(**inputs):
    out, _ = run(trace=False, **inputs)
    return out
